# revision 52
# baseline (speedup 1.0000x reference)
"""Multi-head attention (B=2, S=2048, D=256, H=8) on 8 TRN2 NeuronCores.

Sharding: one head per core (tensor/head parallel). Each core holds
Wq/Wk/Wv for its head and the matching row-slice of Wo, computes its
head's attention plus its partial output projection, and a per-batch
ReduceScatter sums the partials across cores; the host concatenates the
8 row-shards into the full output.

On-device layout (per core, t = b*2048 + s in [0, 4096)):
  xT      [128, 2, 4096] x transposed (feature on partitions, 2 d-chunks)
  QT/KT[m][128, 4096]    projected queries/keys transposed (e on partitions)
  V       [128, 32, 256] values in natural token-major layout
Scores are computed transposed (S^T[key, query]) so softmax's key-axis
reduction is a partition reduction done via a per-128-column ones-matmul
on an fp32 accumulator; normalization by 1/rowsum is folded into the
final output-projection copy where the query index is on partitions.
exp() skips max-subtraction: scores ~ N(0,1), so exp stays in range and
softmax(x) is algebraically identical.

Matmuls run in float32r (replicated fp32, ~1e-4 rel err, full PE rate at
free-dim >= 256). Biases bq/bk are added on-device (fused into the
PSUM->SBUF copy); bv/bo commute through the attention/projection and are
added on the host: out += concat(bv) @ Wo + bo.
"""

import sys

sys.path.insert(0, "/opt/trn_rl_repo")

import numpy as np

import concourse.bacc as bacc
import concourse.mybir as mybir
import concourse.tile as tile
from concourse.bass_utils import run_bass_kernel_spmd
from concourse.masks import make_identity

B, S, D, H = 2, 2048, 256, 8
T = B * S  # 4096 tokens
N_CORES = 8
P = 128
NB = 512  # query block size
F32 = mybir.dt.float32
F32R = mybir.dt.float32r

_CACHE = {}


def _build(with_collective=True, repeat=None):
    """repeat=R wraps the whole compute body in a hardware For_i loop (no
    collective) -- used only to measure per-iteration device time from
    wall-clock deltas."""
    nc = bacc.Bacc("TRN2", target_bir_lowering=False, debug=False, num_devices=N_CORES)

    x = nc.dram_tensor("x", [T, D], F32, kind="ExternalInput").ap()
    wq = nc.dram_tensor("wq", [D, D], F32, kind="ExternalInput").ap()
    wk = nc.dram_tensor("wk", [D, D], F32, kind="ExternalInput").ap()
    wv = nc.dram_tensor("wv", [D, D], F32, kind="ExternalInput").ap()
    wo = nc.dram_tensor("wo", [D, D], F32, kind="ExternalInput").ap()
    bq = nc.dram_tensor("bq", [P, 2], F32, kind="ExternalInput").ap()
    bk = nc.dram_tensor("bk", [P, 2], F32, kind="ExternalInput").ap()
    outs = [
        nc.dram_tensor(f"out{b}", [S // N_CORES, D], F32, kind="ExternalOutput").ap()
        for b in range(B)
    ]

    n_tchunk = T // P  # 32 token chunks
    scale = 1.0 / np.sqrt(D)

    with tile.TileContext(nc) as tc:
        with (
            tc.tile_pool(name="big", bufs=1) as big,
            tc.tile_pool(name="wts", bufs=1) as wts,
            tc.tile_pool(name="xin", bufs=8) as xin,
            tc.tile_pool(name="ptile", bufs=6) as ptile,
            tc.tile_pool(name="acc", bufs=3) as accp,
            tc.tile_pool(name="osb", bufs=6) as osbp,
            tc.tile_pool(name="fsb", bufs=4) as fsbp,
            tc.tile_pool(name="small", bufs=1) as small,
            tc.tile_pool(name="recipp", bufs=3) as recipp,
            tc.tile_pool(name="psum", bufs=1, space="PSUM") as psum,
            tc.tile_pool(name="dram", bufs=1, space="DRAM") as dram,
        ):
            # --- constants / weights ---
            ident = small.tile([P, P], F32)
            make_identity(nc, ident)
            ones = small.tile([P, 1], F32)
            nc.vector.memset(ones, 1.0)

            bq_sb = small.tile([P, 2], F32)
            bk_sb = small.tile([P, 2], F32)

            def emit_bias_dmas():
                nc.gpsimd.dma_start(bq_sb[:], bq)
                nc.gpsimd.dma_start(bk_sb[:], bk)

            w_sb = {}
            wr_sb = {}

            def emit_weights():
                # emitted after the first x chunks so the x DMAs (needed
                # immediately by the transposes) go first in the queue
                for name, ap in (("wq", wq), ("wk", wk), ("wv", wv), ("wo", wo)):
                    t_f = wts.tile([P, 2, D], F32, name=f"{name}_f")
                    nc.gpsimd.dma_start(t_f[:], ap.rearrange("(c p) e -> p c e", p=P))
                    t_r = wts.tile([P, 2, D], F32R, name=f"{name}_r")
                    nc.vector.tensor_copy(t_r[:], t_f[:])
                    w_sb[name] = t_f
                    wr_sb[name] = t_r

            # process per batch: b=0 attention overlaps b=1 load/projections
            # (emitted interleaved -- engine queues are in-order), and b=0's
            # ReduceScatter overlaps all of b=1 compute
            xT = big.tile([P, 2, T], F32R, name="xT")
            QT = [big.tile([P, T], F32R, name=f"QT{m}") for m in range(2)]
            KT = [big.tile([P, T], F32R, name=f"KT{m}") for m in range(2)]
            V = big.tile([P, n_tchunk, D], F32R, name="V")
            h_chunk = n_tchunk // B  # 16 token chunks per batch

            def emit_load_transpose4(n0, alt=False):
                for n in range(n0, n0 + 4):
                    x_chunk = xin.tile([P, D], F32, tag="xchunk", name="x_chunk")
                    nc.sync.dma_start(x_chunk[:], x[n * P : (n + 1) * P, :])
                    tp = psum.tile([P, D], F32, tag="mm512", bufs=4, name="tp")
                    for c in range(2):
                        nc.tensor.transpose(
                            tp[:, c * P : (c + 1) * P],
                            x_chunk[:, c * P : (c + 1) * P],
                            ident[:],
                        )
                    nc.vector.tensor_copy(
                        xT[:, :, n * P : (n + 1) * P],
                        tp.rearrange("p (c t) -> p c t", c=2),
                    )

            def emit_qk_proj(n):
                # one 512-token window of Q^T and K^T, bias fused
                for dst, wname, bias in ((QT, "wq", bq_sb), (KT, "wk", bk_sb)):
                    for m in range(2):
                        ps = psum.tile([P, 512], F32, tag="mm512", bufs=4, name="ps")
                        for c in range(2):
                            nc.tensor.matmul(
                                ps[:],
                                wr_sb[wname][:, c, m * P : (m + 1) * P],
                                xT[:, c, n * 512 : (n + 1) * 512],
                                start=(c == 0),
                                stop=(c == 1),
                            )
                        nc.scalar.activation(
                            dst[m][:, n * 512 : (n + 1) * 512],
                            ps[:],
                            mybir.ActivationFunctionType.Identity,
                            bias=bias[:, m : m + 1],
                        )

            def emit_v_proj(n):
                ps = psum.tile([P, D], F32, tag="mm256", bufs=2, name="psv")
                for c in range(2):
                    nc.tensor.matmul(
                        ps[:],
                        xT[:, c, n * P : (n + 1) * P],
                        wr_sb["wv"][:, c, :],
                        start=(c == 0),
                        stop=(c == 1),
                    )
                nc.vector.tensor_copy(V[:, n, :], ps[:])

            def emit_prep(b, group=None, after_loads=None):
                """Emit load+transpose+projections for batch b; group=g emits
                quarter g (of 4) for interleaving into the other batch."""
                groups = range(4) if group is None else [group]
                for g in groups:
                    emit_load_transpose4(b * h_chunk + 4 * g, alt=(group is None))
                    if after_loads is not None and g == groups[0]:
                        after_loads()
                        after_loads = None
                    emit_qk_proj(b * (S // 512) + g)
                    for n in range(b * h_chunk + 4 * g, b * h_chunk + 4 * (g + 1)):
                        emit_v_proj(n)

            def group_hooks(b, g, jl, jq, jv):
                """(j -> action) pieces of prep group g of batch b."""
                return [
                    (jl, lambda: emit_load_transpose4(b * h_chunk + 4 * g)),
                    (jq, lambda: emit_qk_proj(b * (S // 512) + g)),
                    (
                        jv,
                        lambda: [
                            emit_v_proj(n)
                            for n in range(
                                b * h_chunk + 4 * g, b * h_chunk + 4 * (g + 1)
                            )
                        ],
                    ),
                ]

            def prep_hooks(b, ib):
                """Prep work spread through block (b, ib)'s j-loop. Block
                (0,0) carries batch 0's own groups 1-3 (its j-loop consumes
                KT/V windows in order, so group g must be emitted before
                j=4g); blocks (0,1)/(0,2) carry batch 1's four groups."""
                # group g's qk/v projections MUST be emitted before j=4g --
                # the j-loop reads those KT/V windows starting at j=4g, and
                # emission order is program order
                plan = {
                    (0, 0): group_hooks(0, 1, 1, 2, 3)
                    + group_hooks(0, 2, 4, 6, 7)
                    + group_hooks(0, 3, 8, 10, 11),
                    (0, 1): group_hooks(1, 0, 1, 5, 9) + group_hooks(1, 1, 3, 7, 11),
                    (0, 2): group_hooks(1, 2, 1, 5, 9) + group_hooks(1, 3, 3, 7, 11),
                }
                hooks = {}
                for j, fn in plan.get((b, ib), []):
                    hooks.setdefault(j, []).append(fn)
                return hooks

            import contextlib

            loop_ctx = (
                tc.For_i(0, repeat, 1) if repeat is not None else contextlib.nullcontext()
            )
            with loop_ctx:
                def _first_loads():
                    emit_weights()
                    emit_bias_dmas()

                _emit_body(
                    nc, tc, lambda b, group=None: emit_prep(
                        b, group=group,
                        after_loads=_first_loads if (b == 0 and group == 0) else None,
                    ),
                    prep_hooks,
                    dram, psum, ptile, accp, osbp, fsbp, recipp,
                    QT, KT, V, wr_sb, ones, outs, scale, with_collective,
                )

    nc.compile()
    return nc


def _emit_body(
    nc, tc, emit_prep, prep_hooks, dram, psum, ptile, accp, osbp, fsbp, recipp,
    QT, KT, V, wr_sb, ones, outs, scale, with_collective,
):
    bounces = [dram.tile([S, D], F32, name=f"bounce{b}") for b in range(B)]

    def emit_rs(b):
        # sum partial projections across cores; core r keeps rows
        # [r*256, (r+1)*256) of this batch
        if with_collective:
            rs_out = dram.tile([S // N_CORES, D], F32, name=f"rs{b}")
            nc.gpsimd.collective_compute(
                "ReduceScatter",
                mybir.AluOpType.add,
                replica_groups=[list(range(N_CORES))],
                ins=[bounces[b][:].opt()],
                outs=[rs_out[:].opt()],
            )
            nc.sync.dma_start(outs[b], rs_out[:])
        else:  # single-core cost-model sim: skip the collective
            nc.sync.dma_start(outs[b], bounces[b][0 : S // N_CORES, :])

    def make_epilogue(b, ib, acc, o_sb):
        # deferred: emitted a few j-iterations into the NEXT block so the
        # acc chain and O copies are long done when PE reaches these
        def epilogue():
            # column sums of exp(S^T) -> per-query softmax denominators
            sums_ps = psum.tile(
                [P, 4], F32, tag="mm256", bufs=2, padded_shape=[P, D], name="sums_ps"
            )
            for q in range(NB // P):
                nc.tensor.matmul(
                    sums_ps[:, q : q + 1],
                    acc[:, q * P : (q + 1) * P],
                    ones[:],
                    start=True,
                    stop=True,
                )
            recip = recipp.tile([P, 4], F32, tag="recip", name="recip")
            nc.vector.reciprocal(recip[:], sums_ps[:])
            for q in range(NB // P):
                f_ps = psum.tile([P, D], F32, tag="mm256", bufs=2, name="f_ps")
                for m in range(2):
                    nc.tensor.matmul(
                        f_ps[:],
                        o_sb[m][:, q * P : (q + 1) * P],
                        wr_sb["wo"][:, m, :],
                        start=(m == 0),
                        stop=(m == 1),
                    )
                f_sb = fsbp.tile([P, D], F32, tag="f", name="f_sb")
                nc.scalar.activation(
                    f_sb[:],
                    f_ps[:],
                    mybir.ActivationFunctionType.Copy,
                    scale=recip[:, q : q + 1],
                )
                r0 = ib * NB + q * P
                nc.sync.dma_start(bounces[b][r0 : r0 + P, :], f_sb[:])
            if ib == S // NB - 1:
                emit_rs(b)

        return epilogue

    emit_prep(0, group=0)
    pending = None
    for b in range(B):
        for ib in range(S // NB):
            hooks = prep_hooks(b, ib)
            i0 = b * S + ib * NB
            o_ps = [
                psum.tile([P, NB], F32, tag="opsum", name=f"ops{m}", bufs=2)
                for m in range(2)
            ]
            acc = accp.tile([P, NB], F32, tag="acc")
            for j in range(S // P):
                if j == 2 and pending is not None:
                    pending()
                    pending = None
                for fn in hooks.get(j, ()):
                    fn()
                jc = b * (S // P) + j
                s_ps = psum.tile([P, NB], F32, tag="mm512", bufs=4)
                for m in range(2):
                    nc.tensor.matmul(
                        s_ps[:],
                        KT[m][:, jc * P : (jc + 1) * P],
                        QT[m][:, i0 : i0 + NB],
                        start=(m == 0),
                        stop=(m == 1),
                    )
                p_sb = ptile.tile([P, NB], F32R, tag="p")
                nc.scalar.activation(
                    p_sb[:], s_ps[:], mybir.ActivationFunctionType.Exp, scale=scale
                )
                if j == 0:
                    nc.vector.tensor_copy(acc[:], p_sb[:])
                else:
                    nc.vector.tensor_add(acc[:], acc[:], p_sb[:])
                for m in range(2):
                    nc.tensor.matmul(
                        o_ps[m][:],
                        V[:, jc, m * P : (m + 1) * P],
                        p_sb[:],
                        start=(j == 0),
                        stop=(j == S // P - 1),
                    )
            # copy O out of PSUM immediately (frees the opsum slots for the
            # next block); everything needing acc/sums is deferred
            o_sb = [
                osbp.tile([P, NB], F32R, tag=f"osb{m}", name=f"osb{m}")
                for m in range(2)
            ]
            nc.scalar.copy(o_sb[0][:], o_ps[0][:])
            nc.vector.tensor_copy(o_sb[1][:], o_ps[1][:])
            pending = make_epilogue(b, ib, acc, o_sb)
    pending()


def kernel(x, Wq, bq, Wk, bk, Wv, bv, Wo, bo):
    x = np.ascontiguousarray(np.asarray(x, dtype=np.float32))
    Wq, Wk, Wv = (np.asarray(w, dtype=np.float32) for w in (Wq, Wk, Wv))
    Wo = np.asarray(Wo, dtype=np.float32)
    bq, bk, bv, bo = (np.asarray(v, dtype=np.float32) for v in (bq, bk, bv, bo))

    if "nc" not in _CACHE:
        _CACHE["nc"] = _build()
    nc = _CACHE["nc"]

    x_flat = x.reshape(T, D)
    in_maps = []
    for h in range(N_CORES):
        in_maps.append(
            {
                "x": x_flat,
                "wq": np.ascontiguousarray(Wq[h]),
                "wk": np.ascontiguousarray(Wk[h]),
                "wv": np.ascontiguousarray(Wv[h]),
                "wo": np.ascontiguousarray(Wo[h * D : (h + 1) * D, :]),
                "bq": np.ascontiguousarray(bq[h].reshape(2, P).T),
                "bk": np.ascontiguousarray(bk[h].reshape(2, P).T),
            }
        )

    res = run_bass_kernel_spmd(nc, in_maps, core_ids=list(range(N_CORES)))

    out = np.empty((B, S, D), dtype=np.float32)
    rows = S // N_CORES
    for r in range(N_CORES):
        for b in range(B):
            out[b, r * rows : (r + 1) * rows, :] = res.results[r][f"out{b}"]

    # bv and bo commute through attention (softmax rows sum to 1) and the
    # output projection; fold them here.
    extra = bv.reshape(H * D).astype(np.float64) @ Wo.astype(np.float64) + bo
    out += extra.astype(np.float32)[None, None, :]
    return out
